# revision 22
# baseline (speedup 1.0000x reference)
"""Ising simulated-annealing sampler on 8 TRN2 NeuronCores, pure data parallel.

Reformulation (exact): reference flip rule
    accept = u < exp(-beta*dE), dE = -2*s*local, flip = accept & mask
is equivalent to  s_new = sign(s*t - local)  with t = log(u)/(2*beta) when
masked else +3e4 (never flip), local = theta + s@Jsym.

PSUM accumulates z = p - local directly per i-tile (p = spin*t enters via
an fp16 identity matmul; -theta via an f32r identity; -J via fp16 weight
blocks), so the flip update is ONE op per tile, running on two engines in
parallel:
  tile0: spins in {+.5,-.5};  m' = (z>=0)-0.5   DVE tensor_scalar (PSUM)
  tile1: spins in {+1,-1};    s' = Sign(z)      ACT activation     (PSUM)
The per-tile coding is absorbed host-side: jb0 weight blocks are -2*J,
jb1 blocks are -J, and the streamed thresholds are 2t for tile0 rows and
t for tile1 rows.  p = m*(2t) = s*t exactly in fp16 either way.

Per core (B=512 rows), layout [i=256 -> 2x128 partitions, b=512 free].
t streamed fp16 (52MB/core, one DMA/sweep on the SP HWDGE queue) with
the exact jax threefry streams generated host-side.
"""
import numpy as np

NUM_SWEEPS = 200
BETA_MIN = 0.1
BETA_MAX = 5.0
B, N = 4096, 256
NCORES = 8
BC = B // NCORES          # 512 batch rows per core
RING = 8                  # t-stream SBUF ring depth (sweeps)

_CACHED = {}


def _host_random_t(thetas_dtype):
    """Exact jax threefry streams on CPU: s0 [B,N] ±1, u [200,B,N] fp16.

    u = 2t for i<128 (tile0, half-coded spins), t for i>=128 (tile1).
    """
    import jax
    import jax.numpy as jnp

    cpu = jax.devices("cpu")[0]
    with jax.default_device(cpu):
        key = jax.random.key(42)
        k0, kloop = jax.random.split(key)
        s0 = np.where(np.asarray(jax.random.bernoulli(k0, 0.5, (B, N))), 1.0, -1.0).astype(np.float32)
        betas = np.asarray(jnp.geomspace(BETA_MIN, BETA_MAX, NUM_SWEEPS).astype(jnp.float32))

        @jax.jit
        def step(k):
            k, k1, k2 = jax.random.split(k, 3)
            u = jax.random.uniform(k1, (B, N), dtype=jnp.float32)
            m = jax.random.bernoulli(k2, 0.5, (B, N))
            return k, u, m

        u_all = np.empty((NUM_SWEEPS, B, N), dtype=np.float16)
        k = kloop
        for i in range(NUM_SWEEPS):
            k, u, m = step(k)
            u = np.asarray(u, dtype=np.float64)
            m = np.asarray(m)
            with np.errstate(divide="ignore"):
                t2 = np.log(u) / np.float64(betas[i])     # 2t
            t2 = np.clip(t2, -60000.0, 60000.0)
            t2 = np.where(m, t2, 60000.0)
            t2[:, 128:] *= 0.5                            # tile1 rows carry t
            u_all[i] = t2.astype(np.float16)
    return s0, u_all


def _build_nc():
    import concourse.bass as bass
    from concourse import mybir

    f32 = mybir.dt.float32
    f32r = mybir.dt.float32r
    fp16 = mybir.dt.float16

    nc = bass.Bass()
    # DRAM params
    wj_d = nc.declare_dram_parameter("wj", [4, 128, 128], fp16, isOutput=False)   # jb0: -2J, jb1: -J; idx = jb*2+it
    th_d = nc.declare_dram_parameter("th", [2, 128, 512], f32r, isOutput=False)   # -theta^T tiles (pre-rounded f32r)
    th2_d = nc.declare_dram_parameter("th2", [2, 128, 512], f32, isOutput=False)  # +theta (energy tail)
    id_d = nc.declare_dram_parameter("ident", [128, 128], f32r, isOutput=False)
    id16_d = nc.declare_dram_parameter("ident16", [128, 128], fp16, isOutput=False)
    ones_d = nc.declare_dram_parameter("ones", [128, 1], f32, isOutput=False)
    m0_d = nc.declare_dram_parameter("m0", [2, 128, 512], fp16, isOutput=False)   # tile0: s0/2, tile1: s0
    t_d = nc.declare_dram_parameter("tstream", [NUM_SWEEPS, 128, 1024], fp16, isOutput=False)
    e_d = nc.declare_dram_parameter("energy", [1, 512], f32, isOutput=True)

    # SBUF
    wj_sb = nc.alloc_sbuf_tensor("wj_sb", [128, 4 * 128], fp16).ap()
    th_sb = nc.alloc_sbuf_tensor("th_sb", [128, 1024], f32r).ap()
    th2_sb = nc.alloc_sbuf_tensor("th2_sb", [128, 1024], f32).ap()
    id_sb = nc.alloc_sbuf_tensor("id_sb", [128, 128], f32r).ap()
    id16_sb = nc.alloc_sbuf_tensor("id16_sb", [128, 128], fp16).ap()
    ones_sb = nc.alloc_sbuf_tensor("ones_sb", [128, 1], f32).ap()
    mf = nc.alloc_sbuf_tensor("mf", [128, 2 * 1024], fp16).ap()      # parity x [it0 {±.5} | it1 {±1}]
    tring = nc.alloc_sbuf_tensor("tring", [128, RING * 1024], fp16).ap()
    pbuf = nc.alloc_sbuf_tensor("pbuf", [128, 2 * 1024], fp16).ap()  # parity x [it0|it1]
    sfin = nc.alloc_sbuf_tensor("sfin", [128, 1024], fp16).ap()
    zbuf = nc.alloc_sbuf_tensor("zbuf", [128, 1024], f32).ap()
    ybuf = nc.alloc_sbuf_tensor("ybuf", [128, 1024], f32).ap()
    eout = nc.alloc_sbuf_tensor("eout", [1, 512], f32).ap()
    # PSUM: 4 banks sweep accumulators (parity x 2 tiles) + 1 bank energy
    acc = nc.alloc_psum_tensor("acc", [128, 2048], f32).ap()
    scr = nc.alloc_psum_tensor("scr", [128, 512], f32).ap()

    NCONST = 13 * 16   # 13 const DMAs x16
    with (
        nc.Block() as block,
        nc.semaphore("sem_const") as sem_const,
        nc.semaphore("sem_t") as sem_t,
        nc.semaphore("sem_p0") as sem_p0,
        nc.semaphore("sem_p1") as sem_p1,
        nc.semaphore("sem_m0") as sem_m0,
        nc.semaphore("sem_m1") as sem_m1,
        nc.semaphore("sem_s0") as sem_s0,
        nc.semaphore("sem_s1") as sem_s1,
        nc.semaphore("sem_sf") as sem_sf,
        nc.semaphore("sem_y") as sem_y,
        nc.semaphore("sem_out") as sem_out,
        nc.semaphore("sem_dma") as sem_dma,
    ):
        @block.sync
        def _(eng):
            # const loads
            for i in range(4):
                eng.dma_start(out=wj_sb[:, i * 128:(i + 1) * 128], in_=wj_d[i]).then_inc(sem_const, 16)
            for i in range(2):
                eng.dma_start(out=th_sb[:, i * 512:(i + 1) * 512], in_=th_d[i]).then_inc(sem_const, 16)
            for i in range(2):
                eng.dma_start(out=th2_sb[:, i * 512:(i + 1) * 512], in_=th2_d[i]).then_inc(sem_const, 16)
            eng.dma_start(out=id_sb[:], in_=id_d[:]).then_inc(sem_const, 16)
            eng.dma_start(out=id16_sb[:], in_=id16_d[:]).then_inc(sem_const, 16)
            eng.dma_start(out=ones_sb[:], in_=ones_d[:]).then_inc(sem_const, 16)
            for i in range(2):
                eng.dma_start(out=mf[:, 1024 + i * 512:1024 + (i + 1) * 512], in_=m0_d[i]).then_inc(sem_const, 16)
            # t stream: one [128,1024] fp16 DMA per sweep
            for k in range(NUM_SWEEPS):
                if k >= RING:
                    eng.wait_ge(sem_s0, k - RING + 1)
                slot = (k % RING) * 1024
                eng.dma_start(out=tring[:, slot:slot + 1024], in_=t_d[k]).then_inc(sem_t, 16)
            # output
            eng.wait_ge(sem_out, 1)
            eng.dma_start(out=e_d[:], in_=eout[:]).then_inc(sem_dma, 16)
            eng.wait_ge(sem_dma, 16)

        @block.tensor
        def _(eng):
            id16 = id16_sb[:]
            wjr = [wj_sb[:, i * 128:(i + 1) * 128] for i in range(4)]
            for k in range(NUM_SWEEPS + 1):
                pb = (k % 2) * 1024
                rp = ((k + 1) % 2) * 1024
                last = k == NUM_SWEEPS
                if k == 0:
                    eng.wait_ge(sem_const, NCONST)
                # bank i group order: theta, jb0 (early thresh), p, jb1 (late, stop)
                for i in range(2):
                    lo = pb + i * 512
                    if k >= 2:
                        eng.wait_ge(sem_s0 if i == 0 else sem_s1, k - 1)   # bank WAR
                    eng.matmul(acc[:, lo:lo + 512], id_sb[:], th_sb[:, i * 512:(i + 1) * 512],
                               start=True, stop=False)
                    eng.wait_ge(sem_s0, k)
                    eng.matmul(acc[:, lo:lo + 512], wjr[0 + i], mf[:, rp:rp + 512],
                               start=False, stop=False)
                    if not last:
                        eng.wait_ge(sem_p0 if i == 0 else sem_p1, k + 1)
                        eng.matmul(acc[:, lo:lo + 512], id16, pbuf[:, pb + i * 512:pb + (i + 1) * 512],
                                   start=False, stop=False)
                    eng.wait_ge(sem_s1, k)
                    eng.matmul(acc[:, lo:lo + 512], wjr[2 + i], mf[:, rp + 512:rp + 1024],
                               start=False, stop=True).then_inc(sem_m0 if i == 0 else sem_m1, 1)
            # energy: ones-matmul over y (fp32)
            eng.wait_ge(sem_y, 1)
            eng.matmul(scr[0:1, :], ones_sb[:], ybuf[:, 0:512], start=True, stop=False)
            eng.matmul(scr[0:1, :], ones_sb[:], ybuf[:, 512:1024], start=False, stop=True).then_inc(sem_m0, 1)

        @block.vector
        def _(eng):
            for k in range(NUM_SWEEPS):
                slot = (k % RING) * 1024
                pp = (k % 2) * 1024
                rp = ((k + 1) % 2) * 1024
                if k == 0:
                    eng.wait_ge(sem_const, NCONST)
                eng.wait_ge(sem_t, 16 * (k + 1))
                # p0 = m0 * 2t, p1 = s1 * t (all fp16 -> fast mode)
                eng.tensor_tensor(pbuf[:, pp:pp + 512], mf[:, rp:rp + 512],
                                  tring[:, slot:slot + 512], op=_op("mult")).then_inc(sem_p0, 1)
                eng.wait_ge(sem_s1, k)
                eng.tensor_tensor(pbuf[:, pp + 512:pp + 1024], mf[:, rp + 512:rp + 1024],
                                  tring[:, slot + 512:slot + 1024], op=_op("mult")).then_inc(sem_p1, 1)
                # tile0 threshold: m'0 = (z0 >= 0) - 0.5 straight from PSUM
                eng.wait_ge(sem_m0, k + 1)
                eng.tensor_scalar(mf[:, pp:pp + 512], acc[:, pp:pp + 512], 0.0, 0.5,
                                  op0=_op("is_ge"), op1=_op("subtract")).then_inc(sem_s0, 1)
            # energy tail: y = (theta - acc) * s_final   (acc = -local)
            rpf = (NUM_SWEEPS % 2) * 1024      # parity of the extra (k=200) group
            eng.wait_ge(sem_m0, NUM_SWEEPS + 1)
            eng.wait_ge(sem_m1, NUM_SWEEPS + 1)
            eng.scalar_tensor_tensor(zbuf[:], acc[:, rpf:rpf + 1024], -1.0, th2_sb[:],
                                     op0=_op("mult"), op1=_op("add"))
            eng.wait_ge(sem_sf, 2)
            eng.tensor_tensor(ybuf[:], zbuf[:], sfin[:], op=_op("mult")).then_inc(sem_y, 1)
            eng.wait_ge(sem_m0, NUM_SWEEPS + 2)
            eng.tensor_copy(eout[:], scr[0:1, :]).then_inc(sem_out, 1)

        @block.scalar
        def _(eng):
            sign = _act("Sign")
            for k in range(NUM_SWEEPS):
                pp = (k % 2) * 1024
                # tile1 threshold: s'1 = Sign(z1) straight from PSUM
                eng.wait_ge(sem_m1, k + 1)
                eng.activation(mf[:, pp + 512:pp + 1024], acc[:, pp + 512:pp + 1024],
                               sign).then_inc(sem_s1, 1)
            # s_final for the energy tail: tile0 is half-coded, tile1 is ±1
            spf = ((NUM_SWEEPS - 1) % 2) * 1024
            eng.wait_ge(sem_s0, NUM_SWEEPS)
            eng.activation(sfin[:, 0:512], mf[:, spf:spf + 512], _act("Copy"),
                           bias=0.0, scale=2.0).then_inc(sem_sf, 1)
            eng.activation(sfin[:, 512:1024], mf[:, spf + 512:spf + 1024], _act("Copy"),
                           bias=0.0, scale=1.0).then_inc(sem_sf, 1)

        @block.gpsimd
        def _(eng):
            pass

    return nc


def _round_f32r(x):
    """Round f32 to the bf16-pair (hi+lo) values the FP32r matmul consumes."""
    import ml_dtypes
    hi = x.astype(ml_dtypes.bfloat16).astype(np.float32)
    lo = (x - hi).astype(ml_dtypes.bfloat16).astype(np.float32)
    return (hi + lo).astype(np.float32)


def _op(name):
    from concourse.alu_op_type import AluOpType
    return getattr(AluOpType, name)


def _act(name):
    from concourse import mybir
    return getattr(mybir.ActivationFunctionType, name)


def kernel(thetas: np.ndarray, gamma: np.ndarray) -> np.ndarray:
    from concourse.bass_utils import run_bass_kernel_spmd

    thetas = np.asarray(thetas, dtype=np.float32)
    gamma = np.asarray(gamma, dtype=np.float32)

    s0, u_all = _host_random_t(thetas.dtype)

    J = np.triu(gamma, 1)
    Jsym = (J + J.T).astype(np.float32)
    Jfp16 = Jsym.astype(np.float16).astype(np.float32)

    # weight blocks: lhsT[k=j, m=i]; idx = jb*2+it.
    # jb0 rows pair with half-coded spins -> -2J; jb1 rows with ±1 spins -> -J.
    wj = np.empty((4, 128, 128), dtype=np.float16)
    for jb in range(2):
        scale = -2.0 if jb == 0 else -1.0
        for it in range(2):
            wj[jb * 2 + it] = (Jfp16[jb * 128:(jb + 1) * 128, it * 128:(it + 1) * 128]
                               * scale).astype(np.float16)
    ident = np.eye(128, dtype=np.float32)
    ident16 = np.eye(128, dtype=np.float16)
    ones = np.ones((128, 1), dtype=np.float32)

    if "nc" not in _CACHED:
        _CACHED["nc"] = _build_nc()
    nc = _CACHED["nc"]

    in_maps = []
    for c_id in range(NCORES):
        sl = slice(c_id * BC, (c_id + 1) * BC)
        thT = np.ascontiguousarray(thetas[sl].T)            # [256, 512] f32
        th2 = np.stack([thT[0:128], thT[128:256]])          # +theta
        th = _round_f32r(-th2)                              # -theta, f32r pre-rounded
        s0T = np.ascontiguousarray(s0[sl].T)                # [256, 512]
        m0 = np.stack([s0T[0:128] * 0.5, s0T[128:256]]).astype(np.float16)
        # u layout: [k][p][it*512+b] = u[k, b, it*128+p]
        tT = u_all[:, sl, :].transpose(0, 2, 1)             # [200, 256, 512]
        tT = tT.reshape(NUM_SWEEPS, 2, 128, 512).transpose(0, 2, 1, 3)
        tT = np.ascontiguousarray(tT.reshape(NUM_SWEEPS, 128, 1024))
        in_maps.append({
            "wj": wj, "th": th, "th2": th2, "ident": ident, "ident16": ident16,
            "ones": ones, "m0": m0, "tstream": tT,
        })

    import os
    kw = {}
    if os.environ.get("ISING_TRACE") == "1":
        kw["trace"] = True
        if os.environ.get("ISING_TRACE_DIR"):
            kw["tmpdir"] = os.environ["ISING_TRACE_DIR"]
    br = run_bass_kernel_spmd(nc, in_maps, list(range(NCORES)), **kw)
    LAST["br"] = br
    res = br.results
    out = np.empty((B,), dtype=np.float32)
    for c_id in range(NCORES):
        out[c_id * BC:(c_id + 1) * BC] = 0.5 * res[c_id]["energy"][0]
    return out


LAST = {}


# revision 33
# speedup vs baseline: 1.0969x; 1.0969x over previous
"""Ising simulated-annealing sampler on 8 TRN2 NeuronCores, pure data parallel.

Reformulation (exact): reference flip rule
    accept = u < exp(-beta*dE), dE = -2*s*local, flip = accept & mask
is equivalent to  s_new = sign(s*t - local)  with t = log(u)/(2*beta) when
masked else +3e4 (never flip), local = theta + s@Jsym.

PSUM accumulates z = p - local directly per i-tile (p = spin*t enters via
an fp16 identity matmul; -theta via an f32r identity; -J via fp16 weight
blocks), so the flip update is ONE op per tile, running on two engines in
parallel:
  tile0: spins in {+.5,-.5};  m' = (z>=0)-0.5   DVE tensor_scalar (PSUM)
  tile1: spins in {+1,-1};    s' = Sign(z)      ACT activation     (PSUM)
The per-tile coding is absorbed host-side: jb0 weight blocks are -2*J,
jb1 blocks are -J, and the streamed thresholds are 2t for tile0 rows and
t for tile1 rows.  p = m*(2t) = s*t exactly in fp16 either way.

Per core (B=512 rows), layout [i=256 -> 2x128 partitions, b=512 free].
t streamed fp16 (52MB/core, one DMA/sweep on the SP HWDGE queue) with
the exact jax threefry streams generated host-side.
"""
import numpy as np

NUM_SWEEPS = 200
BETA_MIN = 0.1
BETA_MAX = 5.0
B, N = 4096, 256
NCORES = 8
BC = B // NCORES          # 512 batch rows per core
RING = 8                  # t-stream SBUF ring depth (sweeps)
BANK_ORDER = ((0, 0), (0, 1), (1, 0), (1, 1))
DVE_ORDER = ("p0", "p1a", "t0a", "p1b", "t0b")

_CACHED = {}


def _host_random_t(thetas_dtype):
    """Exact jax threefry streams on CPU: s0 [B,N] ±1, u [200,B,N] fp16.

    u = 2t for i<128 (tile0, half-coded spins), t for i>=128 (tile1).
    """
    import jax
    import jax.numpy as jnp

    cpu = jax.devices("cpu")[0]
    with jax.default_device(cpu):
        key = jax.random.key(42)
        k0, kloop = jax.random.split(key)
        s0 = np.where(np.asarray(jax.random.bernoulli(k0, 0.5, (B, N))), 1.0, -1.0).astype(np.float32)
        betas = np.asarray(jnp.geomspace(BETA_MIN, BETA_MAX, NUM_SWEEPS).astype(jnp.float32))

        @jax.jit
        def step(k):
            k, k1, k2 = jax.random.split(k, 3)
            u = jax.random.uniform(k1, (B, N), dtype=jnp.float32)
            m = jax.random.bernoulli(k2, 0.5, (B, N))
            return k, u, m

        u_all = np.empty((NUM_SWEEPS, B, N), dtype=np.float16)
        k = kloop
        for i in range(NUM_SWEEPS):
            k, u, m = step(k)
            u = np.asarray(u, dtype=np.float64)
            m = np.asarray(m)
            with np.errstate(divide="ignore"):
                t2 = np.log(u) / np.float64(betas[i])     # 2t
            t2 = np.clip(t2, -60000.0, 60000.0)
            t2 = np.where(m, t2, 60000.0)
            t2[:, 128:] *= 0.5                            # tile1 rows carry t
            u_all[i] = t2.astype(np.float16)
    return s0, u_all


def _build_nc():
    import concourse.bass as bass
    from concourse import mybir

    f32 = mybir.dt.float32
    f32r = mybir.dt.float32r
    fp16 = mybir.dt.float16

    nc = bass.Bass()
    # DRAM params
    wj_d = nc.declare_dram_parameter("wj", [4, 128, 128], fp16, isOutput=False)   # jb0: -2J, jb1: -J; idx = jb*2+it
    th_d = nc.declare_dram_parameter("th", [2, 128, 512], f32r, isOutput=False)   # -theta^T tiles (pre-rounded f32r)
    th2_d = nc.declare_dram_parameter("th2", [2, 128, 512], f32, isOutput=False)  # +theta (energy tail)
    id_d = nc.declare_dram_parameter("ident", [128, 128], f32r, isOutput=False)
    id16_d = nc.declare_dram_parameter("ident16", [128, 128], fp16, isOutput=False)
    ones_d = nc.declare_dram_parameter("ones", [128, 1], f32, isOutput=False)
    m0_d = nc.declare_dram_parameter("m0", [2, 128, 512], fp16, isOutput=False)   # tile0: s0/2, tile1: s0
    t_d = nc.declare_dram_parameter("tstream", [NUM_SWEEPS, 128, 1024], fp16, isOutput=False)
    e_d = nc.declare_dram_parameter("energy", [1, 512], f32, isOutput=True)

    # SBUF
    wj_sb = nc.alloc_sbuf_tensor("wj_sb", [128, 4 * 128], fp16).ap()
    th_sb = nc.alloc_sbuf_tensor("th_sb", [128, 1024], f32r).ap()
    th2_sb = nc.alloc_sbuf_tensor("th2_sb", [128, 1024], f32).ap()
    id_sb = nc.alloc_sbuf_tensor("id_sb", [128, 128], f32r).ap()
    id16_sb = nc.alloc_sbuf_tensor("id16_sb", [128, 128], fp16).ap()
    ones_sb = nc.alloc_sbuf_tensor("ones_sb", [128, 1], f32).ap()
    mf = nc.alloc_sbuf_tensor("mf", [128, 2 * 1024], fp16).ap()      # parity x [it0 {±.5} | it1 {±1}]
    tring = nc.alloc_sbuf_tensor("tring", [128, RING * 1024], fp16).ap()
    pbuf = nc.alloc_sbuf_tensor("pbuf", [128, 2 * 1024], fp16).ap()  # parity x [it0|it1]
    sfin = nc.alloc_sbuf_tensor("sfin", [128, 1024], fp16).ap()
    zbuf = nc.alloc_sbuf_tensor("zbuf", [128, 1024], f32).ap()
    ybuf = nc.alloc_sbuf_tensor("ybuf", [128, 1024], f32).ap()
    eout = nc.alloc_sbuf_tensor("eout", [1, 512], f32).ap()
    # PSUM: 8 banks, one per (parity, tile, b-half) quarter; each quarter
    # uses the low half of its bank so accumulation groups never share a
    # bank.  The energy row-sum reuses bank 4 (parity-1 quarter) at the end.
    acc = nc.alloc_psum_tensor("acc", [128, 4096], f32).ap()

    NCONST = 13 * 16   # 13 const DMAs x16
    with (
        nc.Block() as block,
        nc.semaphore("sem_const") as sem_const,
        nc.semaphore("sem_t") as sem_t,
        nc.semaphore("sem_p0") as sem_p0,
        nc.semaphore("sem_p1") as sem_p1,
        nc.semaphore("sem_m0") as sem_m0,
        nc.semaphore("sem_m1") as sem_m1,
        nc.semaphore("sem_s0") as sem_s0,
        nc.semaphore("sem_s1") as sem_s1,
        nc.semaphore("sem_sf") as sem_sf,
        nc.semaphore("sem_y") as sem_y,
        nc.semaphore("sem_out") as sem_out,
        nc.semaphore("sem_dma") as sem_dma,
    ):
        @block.sync
        def _(eng):
            # const loads
            for i in range(4):
                eng.dma_start(out=wj_sb[:, i * 128:(i + 1) * 128], in_=wj_d[i]).then_inc(sem_const, 16)
            for i in range(2):
                eng.dma_start(out=th_sb[:, i * 512:(i + 1) * 512], in_=th_d[i]).then_inc(sem_const, 16)
            for i in range(2):
                eng.dma_start(out=th2_sb[:, i * 512:(i + 1) * 512], in_=th2_d[i]).then_inc(sem_const, 16)
            eng.dma_start(out=id_sb[:], in_=id_d[:]).then_inc(sem_const, 16)
            eng.dma_start(out=id16_sb[:], in_=id16_d[:]).then_inc(sem_const, 16)
            eng.dma_start(out=ones_sb[:], in_=ones_d[:]).then_inc(sem_const, 16)
            for i in range(2):
                eng.dma_start(out=mf[:, 1024 + i * 512:1024 + (i + 1) * 512], in_=m0_d[i]).then_inc(sem_const, 16)
            # t stream: one [128,1024] fp16 DMA per sweep
            for k in range(NUM_SWEEPS):
                if k >= RING:
                    eng.wait_ge(sem_s0, 2 * (k - RING + 1))
                slot = (k % RING) * 1024
                eng.dma_start(out=tring[:, slot:slot + 1024], in_=t_d[k]).then_inc(sem_t, 16)
            # output
            eng.wait_ge(sem_out, 1)
            eng.dma_start(out=e_d[:], in_=eout[:]).then_inc(sem_dma, 16)
            eng.wait_ge(sem_dma, 16)

        @block.tensor
        def _(eng):
            id16 = id16_sb[:]
            wjr = [wj_sb[:, i * 128:(i + 1) * 128] for i in range(4)]
            for k in range(NUM_SWEEPS + 1):
                pb = (k % 2) * 1024
                rp = ((k + 1) % 2) * 1024
                last = k == NUM_SWEEPS
                if k == 0:
                    eng.wait_ge(sem_const, NCONST)
                # Quarter (i, h) groups; order (0,a),(0,b),(1,a),(1,b).
                # The group's dependent J-matmul goes FIRST (start=True) so its
                # wait also covers the quarter's WAR hazard; waits implied by
                # earlier same-sem waits in the sweep are dropped (in-order SEQ).
                for i, h in BANK_ORDER:
                    lo = 2 * pb + (i * 2 + h) * 512
                    bh = h * 256
                    smi = sem_m0 if i == 0 else sem_m1
                    jb0_mm = (wjr[0 + i], mf[:, rp + bh:rp + bh + 256])
                    jb1_mm = (wjr[2 + i], mf[:, rp + 512 + bh:rp + 512 + bh + 256])
                    th_mm = (id_sb[:], th_sb[:, i * 512 + bh:i * 512 + bh + 256])
                    pm_mm = (id16, pbuf[:, pb + i * 512 + bh:pb + i * 512 + bh + 256])
                    if i == 0:
                        eng.wait_ge(sem_s0, max(2 * k + h - 1, 0))
                        eng.matmul(acc[:, lo:lo + 256], *jb0_mm, start=True, stop=False)
                        eng.matmul(acc[:, lo:lo + 256], *th_mm, start=False, stop=False)
                        if not last:
                            if h == 0:
                                eng.wait_ge(sem_p0, k + 1)
                            eng.matmul(acc[:, lo:lo + 256], *pm_mm, start=False, stop=False)
                        eng.wait_ge(sem_s1, max(2 * k + h - 1, 0))
                        eng.matmul(acc[:, lo:lo + 256], *jb1_mm, start=False, stop=True).then_inc(smi, 1)
                    else:
                        # s0/s1 >= 2k already awaited during the tile-0 quarters
                        eng.matmul(acc[:, lo:lo + 256], *jb1_mm, start=True, stop=False)
                        eng.matmul(acc[:, lo:lo + 256], *th_mm, start=False, stop=False)
                        if not last:
                            eng.wait_ge(sem_p1, 2 * k + h + 1)
                            eng.matmul(acc[:, lo:lo + 256], *pm_mm, start=False, stop=False)
                        eng.matmul(acc[:, lo:lo + 256], *jb0_mm, start=False, stop=True).then_inc(smi, 1)
            # energy: ones-matmul over y (fp32)
            eng.wait_ge(sem_y, 1)
            scr = acc[:, 2048:2560]
            eng.matmul(scr[0:1, :], ones_sb[:], ybuf[:, 0:512], start=True, stop=False)
            eng.matmul(scr[0:1, :], ones_sb[:], ybuf[:, 512:1024], start=False, stop=True).then_inc(sem_m0, 1)

        @block.vector
        def _(eng):
            for k in range(NUM_SWEEPS):
                slot = (k % RING) * 1024
                pp = (k % 2) * 1024
                rp = ((k + 1) % 2) * 1024
                if k == 0:
                    eng.wait_ge(sem_const, NCONST)
                eng.wait_ge(sem_t, 16 * (k + 1))

                def p1h(h):
                    eng.wait_ge(sem_s1, max(2 * k - 1 + h, 0))
                    eng.tensor_tensor(pbuf[:, pp + 512 + h * 256:pp + 768 + h * 256],
                                      mf[:, rp + 512 + h * 256:rp + 768 + h * 256],
                                      tring[:, slot + 512 + h * 256:slot + 768 + h * 256],
                                      op=_op("mult")).then_inc(sem_p1, 1)

                def p0m():
                    eng.tensor_tensor(pbuf[:, pp:pp + 512], mf[:, rp:rp + 512],
                                      tring[:, slot:slot + 512], op=_op("mult")).then_inc(sem_p0, 1)

                def t0h(h):
                    q = 2 * pp + h * 512
                    eng.wait_ge(sem_m0, 2 * k + 1 + h)
                    eng.tensor_scalar(mf[:, pp + h * 256:pp + 256 + h * 256],
                                      acc[:, q:q + 256], 0.0, 0.5,
                                      op0=_op("is_ge"), op1=_op("subtract")).then_inc(sem_s0, 1)

                ops = {"p0": p0m, "p1a": lambda: p1h(0), "p1b": lambda: p1h(1),
                       "t0a": lambda: t0h(0), "t0b": lambda: t0h(1)}
                for name in DVE_ORDER:
                    ops[name]()
            # energy tail: y = (theta - acc) * s_final   (acc = -local)
            rpf = (NUM_SWEEPS % 2) * 2048      # parity of the extra (k=200) group
            eng.wait_ge(sem_m0, 2 * NUM_SWEEPS + 2)
            eng.wait_ge(sem_m1, 2 * NUM_SWEEPS + 2)
            for q in range(4):
                eng.scalar_tensor_tensor(zbuf[:, q * 256:(q + 1) * 256],
                                         acc[:, rpf + q * 512:rpf + q * 512 + 256], -1.0,
                                         th2_sb[:, q * 256:(q + 1) * 256],
                                         op0=_op("mult"), op1=_op("add"))
            eng.wait_ge(sem_sf, 2)
            eng.tensor_tensor(ybuf[:], zbuf[:], sfin[:], op=_op("mult")).then_inc(sem_y, 1)
            eng.wait_ge(sem_m0, 2 * NUM_SWEEPS + 3)
            eng.tensor_copy(eout[:], acc[0:1, 2048:2560]).then_inc(sem_out, 1)

        @block.scalar
        def _(eng):
            sign = _act("Sign")
            for k in range(NUM_SWEEPS):
                pp = (k % 2) * 1024
                # tile1 threshold halves: s'1 = Sign(z1) straight from PSUM
                eng.wait_ge(sem_m1, 2 * k + 1)
                eng.activation(mf[:, pp + 512:pp + 768], acc[:, 2 * pp + 1024:2 * pp + 1280],
                               sign).then_inc(sem_s1, 1)
                eng.wait_ge(sem_m1, 2 * k + 2)
                eng.activation(mf[:, pp + 768:pp + 1024], acc[:, 2 * pp + 1536:2 * pp + 1792],
                               sign).then_inc(sem_s1, 1)
            # s_final for the energy tail: tile0 is half-coded, tile1 is ±1
            spf = ((NUM_SWEEPS - 1) % 2) * 1024
            eng.wait_ge(sem_s0, 2 * NUM_SWEEPS)
            eng.activation(sfin[:, 0:512], mf[:, spf:spf + 512], _act("Copy"),
                           bias=0.0, scale=2.0).then_inc(sem_sf, 1)
            eng.activation(sfin[:, 512:1024], mf[:, spf + 512:spf + 1024], _act("Copy"),
                           bias=0.0, scale=1.0).then_inc(sem_sf, 1)

        @block.gpsimd
        def _(eng):
            pass

    return nc


def _round_f32r(x):
    """Round f32 to the bf16-pair (hi+lo) values the FP32r matmul consumes."""
    import ml_dtypes
    hi = x.astype(ml_dtypes.bfloat16).astype(np.float32)
    lo = (x - hi).astype(ml_dtypes.bfloat16).astype(np.float32)
    return (hi + lo).astype(np.float32)


def _op(name):
    from concourse.alu_op_type import AluOpType
    return getattr(AluOpType, name)


def _act(name):
    from concourse import mybir
    return getattr(mybir.ActivationFunctionType, name)


def kernel(thetas: np.ndarray, gamma: np.ndarray) -> np.ndarray:
    from concourse.bass_utils import run_bass_kernel_spmd

    thetas = np.asarray(thetas, dtype=np.float32)
    gamma = np.asarray(gamma, dtype=np.float32)

    s0, u_all = _host_random_t(thetas.dtype)

    J = np.triu(gamma, 1)
    Jsym = (J + J.T).astype(np.float32)
    Jfp16 = Jsym.astype(np.float16).astype(np.float32)

    # weight blocks: lhsT[k=j, m=i]; idx = jb*2+it.
    # jb0 rows pair with half-coded spins -> -2J; jb1 rows with ±1 spins -> -J.
    wj = np.empty((4, 128, 128), dtype=np.float16)
    for jb in range(2):
        scale = -2.0 if jb == 0 else -1.0
        for it in range(2):
            wj[jb * 2 + it] = (Jfp16[jb * 128:(jb + 1) * 128, it * 128:(it + 1) * 128]
                               * scale).astype(np.float16)
    ident = np.eye(128, dtype=np.float32)
    ident16 = np.eye(128, dtype=np.float16)
    ones = np.ones((128, 1), dtype=np.float32)

    if "nc" not in _CACHED:
        _CACHED["nc"] = _build_nc()
    nc = _CACHED["nc"]

    in_maps = []
    for c_id in range(NCORES):
        sl = slice(c_id * BC, (c_id + 1) * BC)
        thT = np.ascontiguousarray(thetas[sl].T)            # [256, 512] f32
        th2 = np.stack([thT[0:128], thT[128:256]])          # +theta
        th = _round_f32r(-th2)                              # -theta, f32r pre-rounded
        s0T = np.ascontiguousarray(s0[sl].T)                # [256, 512]
        m0 = np.stack([s0T[0:128] * 0.5, s0T[128:256]]).astype(np.float16)
        # u layout: [k][p][it*512+b] = u[k, b, it*128+p]
        tT = u_all[:, sl, :].transpose(0, 2, 1)             # [200, 256, 512]
        tT = tT.reshape(NUM_SWEEPS, 2, 128, 512).transpose(0, 2, 1, 3)
        tT = np.ascontiguousarray(tT.reshape(NUM_SWEEPS, 128, 1024))
        in_maps.append({
            "wj": wj, "th": th, "th2": th2, "ident": ident, "ident16": ident16,
            "ones": ones, "m0": m0, "tstream": tT,
        })

    import os
    kw = {}
    if os.environ.get("ISING_TRACE") == "1":
        kw["trace"] = True
        if os.environ.get("ISING_TRACE_DIR"):
            kw["tmpdir"] = os.environ["ISING_TRACE_DIR"]
    br = run_bass_kernel_spmd(nc, in_maps, list(range(NCORES)), **kw)
    LAST["br"] = br
    res = br.results
    out = np.empty((B,), dtype=np.float32)
    for c_id in range(NCORES):
        out[c_id * BC:(c_id + 1) * BC] = 0.5 * res[c_id]["energy"][0]
    return out


LAST = {}
